# revision 19
# baseline (speedup 1.0000x reference)
"""Trainium2 Bass kernel: 2-layer LSTM (B=16384, T=28, IN=28, H=84) + Linear head.

Strategy (pure data parallel over 8 NeuronCores, 2048 batch rows each):
  - Transposed layout on-chip: states are [hidden, batch] so the tiny LSTM
    weights are the PE-stationary operand and the 2048-wide batch streams.
  - Pure-gate M-chunks (84 rows): SBUF engine-op access patterns must start
    at partition 0/32/64/96 (32/96 -> <=32 rows, 64 -> <=64), so every
    84-row gate tensor lives at partition 0 of its own tile.
  - Layer 0 streams rhs z = [x_t(0:28) | h(28:112)] (K=112): input +
    recurrent projections in one matmul per gate.  Both z parts arrive by
    DMA (exempt from the partition-start rule): x_t from DRAM scratch
    xT[T, 28, BC], h as an SBUF->SBUF copy of the GPSIMD-written hbuf,
    issued on the GPSIMD queue so it needs no extra semaphore.
  - Walrus caps sync-waits per instruction (ACT: 1, PE matmul: 2, PE
    ldweights: 1; DVE/DMA are roomier).  The schedule keeps every
    instruction within budget: "lightning-rod" ldweights (bf16 bitcast
    views; fp32 ldweights is unsupported) absorb DMA semaphores before the
    matmuls, the whole cell runs on DVE so ACT's single cross-engine wait
    (tanh(c) -> DVE) transitively covers all s-tile WARs, and all DMAs are
    issued from the sync queue.
  - (b_ih+b_hh) biases fold into the PSUM->SBUF activation eviction.
  - x arrives host-pre-transposed as xT[T*IN, BC]; h1 is spilled to DRAM
    between the layers, each slice written by a single DMA so consumers
    need only one wait.
Output is produced transposed ([10, BC] per core) and fixed up on host.
"""

import os
import sys

for _p in ("/opt/trn_rl_repo", "/root/.axon_site/_ro/trn_rl_repo"):
    if os.path.isdir(_p) and _p not in sys.path:
        sys.path.insert(0, _p)

import numpy as np

import concourse.bass as bass
import concourse.mybir as mybir
from concourse import tile
import concourse.tile_sem_assignment as _tsa
from concourse.bass_utils import run_bass_kernel_spmd

try:
    import orjson as _oj
    _loads, _dumps = _oj.loads, _oj.dumps
except ImportError:
    import json as _json
    _loads = _json.loads
    _dumps = lambda d: _json.dumps(d).encode()

# Walrus allows only ONE sync-wait per hardware instruction.  Tile emits as
# many waits as the dependency structure needs, so this post-pass moves every
# excess wait onto an injected same-queue NoOp placed just before the
# instruction (barrier-class instructions take a vector of waits natively).
_WAIT_EXEMPT = set()


def _split_waits(bir: bytes) -> bytes:
    d = _loads(bir)
    n = 0
    for f in d.get("functions", []):
        for bb in f.get("blocks", []):
            out = []
            for inst in bb.get("instructions", []):
                si = inst.get("sync_info") or {}
                w = si.get("on_wait") or []
                if len(w) > 1 and inst.get("opcode") not in _WAIT_EXEMPT:
                    for extra in w[:-1]:
                        n += 1
                        out.append({
                            "debug": inst.get("debug", 0),
                            "engine": inst["engine"],
                            "ins": [], "outs": [],
                            "name": f"WSP-{n}",
                            "opcode": "NoOp",
                            "text_hint": "wsplit",
                            "sync_info": {"on_update": [], "on_wait": [extra]},
                        })
                    si["on_wait"] = [w[-1]]
                out.append(inst)
            bb["instructions"] = out
    return _dumps(d)


class _SplitWaitBass:
    """Delegating Bass wrapper whose to_json_bytes applies _split_waits."""

    def __init__(self, nc):
        object.__setattr__(self, "_nc", nc)

    def __getattr__(self, k):
        return getattr(object.__getattribute__(self, "_nc"), k)

    def to_json_bytes(self):
        return _split_waits(object.__getattribute__(self, "_nc").to_json_bytes())


# All DMAs are issued from the sync queue (one HWDGE ring, FIFO execution).
# Collapsing Tile's 8 round-robin HWDGE semaphore lanes to 1 makes every
# DMA-to-DMA dependency same-proc (no semaphore wait emitted) and every
# consumer's DMA dependency a single semaphore -- walrus allows only one
# sync-wait per instruction.
_tsa.NUM_HWDGE_SEMS = 1

B, T, IN, H, OUT = 16384, 28, 28, 84, 10
KZ = IN + H    # 112: z rows = x(0:28) | h(28:112)
NCORES = 8
F32 = mybir.dt.float32
BF16 = mybir.dt.bfloat16
AF = mybir.ActivationFunctionType
GATES = ("i", "f", "g", "o")  # PyTorch gate row order
ZCOL = 12 * H + 9 + OUT  # start of an all-zero region of wpk
WPK_COLS = ZCOL + 128     # packed: wz|wi1|wh1|b0|b1|wo|bo|zeros


def build(BC: int) -> bass.Bass:
    """Emit the per-core kernel for a batch shard of BC rows (BC % 512 == 0)."""
    assert BC % 512 == 0
    NS = BC // 512    # 512-wide column slices per matmul group
    NCG = BC // 512   # chunk groups of 4x128 batch rows
    NCH = BC // 128   # 128-row batch chunks

    nc = bass.Bass()

    xT = nc.declare_dram_parameter("xT", [T * IN, BC], F32, isOutput=False)
    wpk = nc.declare_dram_parameter("wpk", [128, WPK_COLS], F32, isOutput=False)
    outT = nc.declare_dram_parameter("outT", [OUT, BC], F32, isOutput=True)

    with tile.TileContext(nc) as tc:
        with (
            tc.tile_pool(name="wp", bufs=1) as wp,
            tc.tile_pool(name="dp", bufs=1, space="DRAM") as dp,
        ):
            wpk_t = wp.tile([128, WPK_COLS], F32, name="wpk_t")
            nc.sync.dma_start(wpk_t[:], wpk[:])
            rodA = wp.tile([1, 2], F32, name="rodA")
            nc.scalar.activation(rodA[:], wpk_t[0:1, 0:2], AF.Copy)
            W = {}
            for k, g in enumerate(GATES):
                W[f"wz{g}"] = wpk_t[0:KZ, k * H:(k + 1) * H]
                W[f"wi1{g}"] = wpk_t[0:H, 4 * H + k * H:4 * H + (k + 1) * H]
                W[f"wh1{g}"] = wpk_t[0:H, 8 * H + k * H:8 * H + (k + 1) * H]
                W[f"b0{g}"] = wpk_t[0:H, 12 * H + k:12 * H + k + 1]
                W[f"b1{g}"] = wpk_t[0:H, 12 * H + 4 + k:12 * H + 5 + k]
            wo_t = wpk_t[0:H, 12 * H + 8:12 * H + 8 + OUT]
            bo_t = wpk_t[0:OUT, 12 * H + 8 + OUT:12 * H + 9 + OUT]
            ident_t = wpk_t[0:128, 12 * H + 9 + OUT:12 * H + 9 + OUT + 128]

            h1_d = dp.tile([T, H, BC], F32)

            def rod(ap):
                # absorb one pending semaphore into the PE clock
                nc.tensor.ldweights(ap.bitcast(BF16))

            # ---- Phases L0 / L1 / OUT ----
            with (
                tc.tile_pool(name="gp", bufs=2, space="PSUM") as gp,
                tc.tile_pool(name="sp", bufs=1) as sp,
                tc.tile_pool(name="st", bufs=2) as st,
                tc.tile_pool(name="zp", bufs=3) as zp,
                tc.tile_pool(name="h1p", bufs=3) as h1p,
            ):
                zbias = wpk_t[0:H, ZCOL:ZCOL + 1]  # known-zero column
                b0 = {g: W[f"b0{g}"] for g in GATES}
                b1 = {g: W[f"b1{g}"] for g in GATES}
                slot_user = {0: None, 1: None}  # psum slot -> s-tile of last reader

                def gates_and_sigmas(t, lname, bias, mm_emit):
                    """Per gate: rod(prev slot user) -> matmuls -> sigma/tanh.

                    Interleaving keeps every matmul's psum-slot WAR inside PE's
                    clock (the rod reads the previous user's s-tile, which
                    carries the same ACT semaphore value).
                    """
                    s = {}
                    for k, g in enumerate(GATES):
                        prev = slot_user[k % 2]
                        if prev is not None:
                            rod(prev[0:32, 0:2])
                        psg = gp.tile([H, BC], F32, tag="gate", name=f"p{g}{lname}{t}")
                        mm_emit(g, psg)
                        nb = 2 if g == "o" else 1
                        sg = sp.tile([H, BC], F32, tag=f"s{g}", bufs=nb,
                                     name=f"s{g}{lname}{t}")
                        fn = AF.Tanh if g == "g" else AF.Sigmoid
                        nc.scalar.activation(sg[:], psg[:], fn, bias=bias[g])
                        s[g] = sg
                        slot_user[k % 2] = sg
                    return s

                def cell_rest(t, s, c_prev, hdst, lname, war_rod=None):
                    """DVE cell math from the s-tiles; h -> hdst (SBUF).

                    All elementwise ops on DVE: ACT's only cross-engine wait is
                    tanh(c) -> DVE, which transitively covers the s-tile WARs.
                    war_rod: dead tile whose slot the h-op is about to reuse;
                    a 2-element DVE write absorbs its reader semaphore first.
                    """
                    c_new = st.tile([H, BC], F32, tag="c", name=f"c{lname}{t}")
                    if c_prev is None:
                        nc.vector.tensor_mul(c_new[:], s["i"][:], s["g"][:])
                    else:
                        tmpf = sp.tile([H, BC], F32, tag="tmpf", name=f"tf{lname}{t}")
                        tmpi = sp.tile([H, BC], F32, tag="tmpi", name=f"ti{lname}{t}")
                        nc.vector.tensor_mul(tmpf[:], s["f"][:], c_prev[:])
                        nc.vector.tensor_mul(tmpi[:], s["i"][:], s["g"][:])
                        nc.vector.tensor_add(c_new[:], tmpf[:], tmpi[:])
                    tC = st.tile([H, BC], F32, tag="tC", bufs=1, name=f"tC{lname}{t}")
                    nc.scalar.activation(tC[:], c_new[:], AF.Tanh, bias=zbias)
                    if war_rod is not None:
                        nc.vector.tensor_copy(war_rod[0:1, 0:2], wpk_t[0:1, 0:2])
                    nc.vector.tensor_mul(hdst, s["o"][:], tC[:])
                    return c_new

                # ---- Layer 0 ----
                z = zp.tile([KZ, BC], F32, tag="z", name="z0")
                nc.sync.dma_start(z[0:IN, :], xT[0:IN, :])
                c_prev = None
                hist = []
                for t in range(T):
                    if t == 0:
                        rod(z[0:IN, 0:2])      # absorb x-DMA wait
                    else:
                        # one DMAHW lane: reading all z rows waits on the
                        # latest writer (the h-copy), covering the x-DMA too
                        rod(z[0:KZ, 0:2])
                    zc = z

                    def mm_l0(g, psg):
                        for s_ in range(NS):
                            sl = slice(s_ * 512, (s_ + 1) * 512)
                            if t == 0:
                                nc.tensor.matmul(psg[:, sl], W[f"wz{g}"][0:IN, :],
                                                 zc[0:IN, sl], start=True, stop=True)
                            else:
                                nc.tensor.matmul(psg[:, sl], W[f"wz{g}"],
                                                 zc[:, sl], start=True, stop=True)

                    s = gates_and_sigmas(t, "a", b0, mm_l0)
                    if t < T - 1:
                        z_next = zp.tile([KZ, BC], F32, tag="z", name=f"z{t + 1}")
                        nc.sync.dma_start(z_next[0:IN, :],
                                          xT[(t + 1) * IN:(t + 2) * IN, :])
                    else:
                        z_next = None
                    hbuf = st.tile([H, BC], F32, tag="hbuf", name=f"hb{t}")
                    war = hist[-2] if len(hist) >= 2 else None
                    c_prev = cell_rest(t, s, c_prev, hbuf[:], "a", war)
                    hist.append(hbuf)
                    nc.sync.dma_start(h1_d[t], hbuf[:])
                    if z_next is not None:
                        nc.sync.dma_start(z_next[IN:KZ, :], hbuf[:])
                    z = z_next

                # ---- Layer 1 ----
                c_prev = None
                h2_prev = None
                hist = []
                for t in range(T):
                    h1t = h1p.tile([H, BC], F32, tag="h1t", name=f"h1t{t}")
                    nc.sync.dma_start(h1t[:], h1_d[t])
                    rod(h1t[0:32, 0:2])        # absorb h1-DMA wait
                    h2p = h2_prev

                    def mm_l1(g, psg):
                        for s_ in range(NS):
                            sl = slice(s_ * 512, (s_ + 1) * 512)
                            nc.tensor.matmul(psg[:, sl], W[f"wi1{g}"], h1t[:, sl],
                                             start=True, stop=(t == 0))
                            if t > 0:
                                nc.tensor.matmul(psg[:, sl], W[f"wh1{g}"],
                                                 h2p[:, sl], start=False, stop=True)

                    s = gates_and_sigmas(t, "b", b1, mm_l1)
                    h2_new = st.tile([H, BC], F32, tag="h2", name=f"h2_{t}")
                    war = hist[-2] if len(hist) >= 2 else None
                    c_prev = cell_rest(t, s, c_prev, h2_new[:], "b", war)
                    hist.append(h2_new)
                    h2_prev = h2_new

                # ---- Output projection (emitted transposed; host fixes up) ----
                po = gp.tile([OUT, BC], F32, tag="gate", name="po")
                prev = slot_user[0]
                if prev is not None:
                    rod(prev[0:32, 0:2])
                rod(h2_prev[0:32, 0:2])
                for s_ in range(NS):
                    sl = slice(s_ * 512, (s_ + 1) * 512)
                    nc.tensor.matmul(po[:, sl], wo_t, h2_prev[:, sl], start=True, stop=True)
                so = sp.tile([OUT, BC], F32, tag="oview", name="so")
                nc.vector.tensor_scalar_add(so[:], po[:], bo_t)
                nc.sync.dma_start(outT[:], so[:])

    return nc


def prep_common(w_ih_l0, w_hh_l0, b_ih_l0, b_hh_l0,
                w_ih_l1, w_hh_l1, b_ih_l1, b_hh_l1, w_out, b_out):
    f32 = lambda a: np.ascontiguousarray(np.asarray(a, dtype=np.float32))
    w_ih_l0 = np.asarray(w_ih_l0); w_hh_l0 = np.asarray(w_hh_l0)
    w_ih_l1 = np.asarray(w_ih_l1); w_hh_l1 = np.asarray(w_hh_l1)
    bias0 = np.asarray(b_ih_l0) + np.asarray(b_hh_l0)
    bias1 = np.asarray(b_ih_l1) + np.asarray(b_hh_l1)
    pk = np.zeros((128, WPK_COLS), np.float32)
    for k, g in enumerate(GATES):
        rows = slice(k * H, (k + 1) * H)
        # z rows = [x | h]  ->  weight rows = [w_ih | w_hh]
        wz = np.concatenate([w_ih_l0[rows], w_hh_l0[rows]], axis=1)
        pk[0:KZ, k * H:(k + 1) * H] = wz.T
        pk[0:H, 4 * H + k * H:4 * H + (k + 1) * H] = np.asarray(w_ih_l1)[rows].T
        pk[0:H, 8 * H + k * H:8 * H + (k + 1) * H] = np.asarray(w_hh_l1)[rows].T
        pk[0:H, 12 * H + k] = bias0[rows]
        pk[0:H, 12 * H + 4 + k] = bias1[rows]
    pk[0:H, 12 * H + 8:12 * H + 8 + OUT] = np.asarray(w_out).T
    pk[0:OUT, 12 * H + 8 + OUT] = np.asarray(b_out)
    return {"wpk": f32(pk)}


def pack_x(x_shard):
    """[BC, T, IN] -> [T*IN, BC] time/feature-major (pre-transposed for the kernel)."""
    BC = x_shard.shape[0]
    return np.ascontiguousarray(
        x_shard.reshape(BC, T * IN).T
    )


_CACHE = {}


def _get_nc():
    if "nc" not in _CACHE:
        _CACHE["nc"] = _SplitWaitBass(build(B // NCORES))
    return _CACHE["nc"]


def kernel(x, w_ih_l0, w_hh_l0, b_ih_l0, b_hh_l0,
           w_ih_l1, w_hh_l1, b_ih_l1, b_hh_l1, w_out, b_out,
           _trace=False, _trace_kwargs=None):
    common = prep_common(w_ih_l0, w_hh_l0, b_ih_l0, b_hh_l0,
                         w_ih_l1, w_hh_l1, b_ih_l1, b_hh_l1, w_out, b_out)
    BC = B // NCORES
    xr = np.asarray(x, dtype=np.float32).reshape(B, T * IN)
    in_maps = [
        {"xT": pack_x(xr[i * BC:(i + 1) * BC]), **common}
        for i in range(NCORES)
    ]
    kw = {}
    if _trace:
        kw = {"trace": True, **(_trace_kwargs or {})}
    res = run_bass_kernel_spmd(_get_nc(), in_maps, list(range(NCORES)), **kw)
    out = np.concatenate(
        [np.asarray(r["outT"], dtype=np.float32).T for r in res.results], axis=0
    )
    _CACHE["last_results"] = res
    return np.ascontiguousarray(out)
